# revision 28
# baseline (speedup 1.0000x reference)
"""Causal GQA attention (S=2048, B=2, HQ=32, HKV=8, D=128) on 8 trn2 cores.

Sharding: the 16 (batch, kv-head) pairs are split 2 per core (data+head
parallel). Each pair carries group=4 query heads -> 8 attention heads/core.

Per head the kernel runs flash-attention style with the q-chunk loop OUTER
and the k-tile loop INNER:

  for q-chunk c (512 wide):                 # o accumulates in ONE psum bank
    for k-tile pair (t0, t1):               # 128-row k tiles, 2 at a time
      S^T(t0), S^T(t1) = (K_t Q_c^T) into a 2-bank psum pair
      P^T pair = exp(S^T pair)              # one 1024-wide ACT instruction
      tri-mask diagonal blocks (DVE)
      T += P^T tiles (DVE, bf16)            # cross-k-tile accumulation
      o += V_t^T P^T(t0), V_t^T P^T(t1)     # PE, accumulate in one bank
    den row = ones^T T                      # ONE 512-col matmul per chunk
    out_c = o * (1/den broadcast)           # DVE + DMA round trip

Everything on the PE runs bf16 (fp32/fp32r matmuls double-pump the array
and trip the power throttler). The one-matmul-per-chunk denominator (vs
one per k-tile) cuts PE streaming by ~1/3; the paired exp halves the
~185ns-per-instruction ACT bubble. Output is stored bf16 and upcast on
host (measured ~4e-3 rel err overall vs the 2e-2 gate).

Host side only re-lays-out data: Q/K are fed pre-transposed [d, s] bf16,
V as [k_local, ktile, d] bf16, and the returned out^T [d, s] bf16 is
transposed back and upcast.
"""

import numpy as np

import concourse.bass as bass
import concourse.mybir as mybir
import concourse.tile as tile
from concourse import bacc, bass_utils
from concourse.masks import make_identity, make_lower_triangular

S, B, HQ, HKV, D = 2048, 2, 32, 8, 128
G = HQ // HKV                      # 4 query heads per kv head
NCORES = 8
NPAIRS = B * HKV                   # 16 (batch, kv-head) pairs
PAIRS_PER_CORE = NPAIRS // NCORES  # 2
HEADS_PER_CORE = PAIRS_PER_CORE * G  # 8
SCALE = 1.0 / float(np.sqrt(D))
QC = 512                           # q-chunk (PSUM bank) width
NQC = S // QC                      # 4
KT = 128                           # k-tile (partition) width
NKT = S // KT                      # 16

F32 = mybir.dt.float32
BF16 = mybir.dt.bfloat16
U32 = mybir.dt.uint32


def emit_core_program(tc, qt, kt, v, recd, ot):
    """Emit the per-core program.

    qt: [HEADS_PER_CORE, D, S] bf16   Q^T per head ([d, q])
    kt: [PAIRS_PER_CORE, D, S] bf16   K^T per pair ([d, k])
    v:  [PAIRS_PER_CORE, 128, NKT*D] bf16  V per pair ([k_local, kt, d])
    recd: [HEADS_PER_CORE, NQC, QC] f32 DRAM scratch for 1/sum rows
    ot: [HEADS_PER_CORE, D, S] bf16  out^T per head ([d, q])
    """
    from contextlib import ExitStack

    nc = tc.nc
    with ExitStack() as ctx:
        _emit_core_program(ctx, tc, nc, qt, kt, v, recd, ot)


def _emit_core_program(ctx, tc, nc, qt, kt, v, recd, ot):
    singles = ctx.enter_context(tc.tile_pool(name="singles", bufs=1))
    kv_pool = ctx.enter_context(tc.tile_pool(name="kv", bufs=2))
    q_pool = ctx.enter_context(tc.tile_pool(name="q", bufs=2))
    pp_pool = ctx.enter_context(tc.tile_pool(name="pp", bufs=6))
    t_pool = ctx.enter_context(tc.tile_pool(name="tt", bufs=3))
    ob_pool = ctx.enter_context(tc.tile_pool(name="ob", bufs=3))
    nrm_pool = ctx.enter_context(tc.tile_pool(name="nrm", bufs=3))
    ps_pair = ctx.enter_context(tc.tile_pool(name="ps_pair", bufs=2, space="PSUM"))
    ps_o = ctx.enter_context(tc.tile_pool(name="ps_o", bufs=3, space="PSUM"))
    ps_sum = ctx.enter_context(tc.tile_pool(name="ps_sum", bufs=1, space="PSUM"))

    # Constants
    # maskb[k, q] = -1e9 where q < k (causal-masked), 0 where q >= k. It is
    # injected into the S^T staging bank by an identity matmul with
    # start=True; the QK matmul then accumulates on top (start=False), so
    # exp(scale*(s - 1e9)) = 0 and the DVE never sits on the PE->ACT->PE
    # critical path.
    # Wide variant [128, 512]: cols [0:384] solid -1e9, cols [384:512] the
    # strict-lower triangle. A diagonal k-tile with in-chunk offset o takes
    # rhs = maskw[:, 384-o:512]: o solid columns then the triangular block.
    # The solid part zeroes (after exp) the staging-bank junk between the
    # two halves of a pair, so T can accumulate the full 1024-wide pair.
    maskwf = singles.tile([128, 4 * KT], F32)
    nc.gpsimd.memset(maskwf[:, 0:3 * KT], -1e9)
    make_lower_triangular(nc, maskwf[:, 3 * KT:4 * KT], val=-1e9, diag=False)
    maskw = singles.tile([128, 4 * KT], BF16)
    nc.scalar.copy(out=maskw[:], in_=maskwf[:])
    identf = singles.tile([128, 128], F32)
    make_identity(nc, identf[:])
    identb = singles.tile([128, 128], BF16)
    nc.scalar.copy(out=identb[:], in_=identf[:])
    onesc = singles.tile([128, 1], BF16)   # ones column (sum-over-k lhsT)
    nc.vector.memset(onesc[:], 1.0)

    # Deferred normalization stages: one closure is popped and emitted at
    # the top of each k-tile-pair iteration, so the slow DMA round trips
    # (recip row -> DRAM -> partition-broadcast) never head-of-line block
    # the DVE queue that feeds T accumulation.
    deferred = []

    def pop_deferred():
        if deferred:
            deferred.pop(0)()

    # The denominator ones-matmul chain of chunk c is emitted in the middle
    # of chunk c+1's FIRST pair (after its QK matmuls): the PE queue is
    # in-order, and the ones chain waits on the DVE T accumulation — emitted
    # at the chunk tail it would head-of-line block the next chunk's QKs.
    pending_ones = []

    def pop_ones():
        if pending_ones:
            pending_ones.pop(0)()

    exp = mybir.ActivationFunctionType.Exp

    for pair in range(PAIRS_PER_CORE):
        kt_sb = kv_pool.tile([D, S], BF16, tag="kt")
        nc.sync.dma_start(out=kt_sb[:], in_=kt[pair])
        v_sb = kv_pool.tile([128, NKT * D], BF16, tag="v")
        nc.gpsimd.dma_start(out=v_sb[:], in_=v[pair])

        for g in range(G):
            head = pair * G + g
            q_sb = q_pool.tile([D, S], BF16)
            nc.sync.dma_start(out=q_sb[:], in_=qt[head])

            # One PSUM sum bank per head; chunk c's denominator row lives at
            # partition 32c (tile_position), so chunk c+1's ones-matmuls
            # never WAR against chunk c's still-draining normalization.
            sum_ps = ps_sum.tile([128, QC], F32, tag="sum")

            for c in range(NQC):
                ntiles = 4 * c + 4
                npairs = ntiles // 2
                qs = q_sb[:, QC * c:QC * (c + 1)]
                o_ps = ps_o.tile([128, QC], F32, tag="o", name=f"o_{head}_{c}")
                tt = t_pool.tile([128, 2 * QC], BF16, tag="T",
                                 name=f"T_{head}_{c}")

                # Normalization tail, deferred + spaced so the DMA round
                # trips never head-of-line block the DVE queue:
                #   A: pull the sums row out of PSUM, DMA-reshape to
                #      [128, 4] so the reciprocal runs 128 lanes wide
                #      (a [1, 512] reciprocal measures ~4us on the DVE!)
                #   B: reciprocal, park in DRAM, partition-broadcast back
                #   C: multiply into the out^T chunk and store
                state = {}

                def stage_a(c=c, state=state, sum_ps=sum_ps):
                    row = slice(32 * c, 32 * c + 1)
                    sr = nrm_pool.tile([128, QC], F32, tag="sumrow")
                    nc.vector.tensor_copy(sr[row, :], sum_ps[row, :])
                    srec = nrm_pool.tile([128, NQC], F32, tag="srec")
                    nc.sync.dma_start(out=srec[:], in_=sr[row, :])
                    state["srec"] = srec

                def stage_b(head=head, c=c, state=state):
                    srec2 = nrm_pool.tile([128, NQC], F32, tag="srec2")
                    nc.vector.reciprocal(out=srec2[:], in_=state.pop("srec")[:])
                    nc.sync.dma_start(out=recd[head, c], in_=srec2[:])
                    bcs = nrm_pool.tile([128, QC], F32, tag="bc")
                    nc.sync.dma_start(
                        out=bcs[:], in_=recd[head, c].partition_broadcast(128))
                    state["bcs"] = bcs

                def stage_c(head=head, c=c, o_ps=o_ps, state=state):
                    bcs = state.pop("bcs")
                    osb = ob_pool.tile([128, QC], BF16)
                    nc.vector.tensor_mul(osb[:], o_ps[:], bcs[:])
                    nc.sync.dma_start(
                        out=ot[head][:, QC * c:QC * (c + 1)], in_=osb[:])

                for j in range(npairs):
                    pop_deferred()
                    t0, t1 = 2 * j, 2 * j + 1
                    o0 = max(0, KT * t0 - QC * c)
                    o1 = max(0, KT * t1 - QC * c)
                    sp = ps_pair.tile([128, 2 * QC], F32, tag="spair")
                    pp = pp_pool.tile([128, 2 * QC], BF16, tag="pp")

                    # S^T = (Q K^T)^T for both k-tiles of the pair. On the
                    # diagonal tiles an identity matmul first deposits the
                    # -1e9 causal mask into the staging bank (start=True
                    # clears the bank + sets has_written on the block), and
                    # the QK matmul accumulates on top of it. The odd tile's
                    # mask extends left over the staging junk [QC : QC+o1]
                    # so exp() zeroes it and T can accumulate 1024 wide.
                    for tti, oo, base in ((t0, o0, 0), (t1, o1, QC)):
                        diag = tti >= 4 * c
                        if diag:
                            mw = oo + KT if base else KT
                            nc.tensor.matmul(
                                out=sp[:, base + oo + KT - mw:base + oo + KT],
                                lhsT=identb[:], rhs=maskw[:, 4 * KT - mw:],
                                start=True, stop=False)
                        nc.tensor.matmul(
                            out=sp[:, base + oo:base + QC],
                            lhsT=kt_sb[:, KT * tti:KT * (tti + 1)],
                            rhs=qs[:, oo:QC], start=not diag, stop=True)

                    # previous chunk's denominator chain goes here, BEHIND
                    # this pair's QKs in the in-order PE queue
                    pop_ones()

                    # one wide exp over the pair (cols [QC+o0 : QC+o1] are
                    # junk from the staging bank; nothing consumes them)
                    nc.scalar.activation(
                        pp[:, o0:2 * QC], sp[:, o0:2 * QC], exp, scale=SCALE)

                    # T accumulation: ONE wide bf16 add per pair (the
                    # staging junk between the halves exps to exactly 0, so
                    # adding the full 1024 is safe; the ~151-cycle DVE fixed
                    # cost is paid once instead of twice). The LAST pair of
                    # the chunk skips T: its denominator contribution goes
                    # through direct ones-matmuls on the exp output below,
                    # so the chunk tail never waits on the DVE chain.
                    last = j == npairs - 1
                    if j == 0:
                        # bf16 copy via int32 bitcast: halves the element
                        # count, so the DVE runs at the fp32-copy 2x rate
                        nc.vector.tensor_copy(
                            tt[:, :].bitcast(U32), pp[:, :].bitcast(U32))
                    elif not last:
                        nc.vector.tensor_add(
                            tt[:, o0:2 * QC], tt[:, o0:2 * QC],
                            pp[:, o0:2 * QC])

                    if last:
                        # denominator: the two T halves fold together via
                        # PSUM accumulation, then the last pair's slices
                        # stream direct from the exp output. Emission is
                        # deferred into the next chunk's first pair.
                        def ones_chain(c=c, tt=tt, pp=pp, o0=o0, o1=o1,
                                       sum_ps=sum_ps, stage_a=stage_a,
                                       stage_b=stage_b, stage_c=stage_c):
                            row = slice(32 * c, 32 * c + 1)
                            tp = (0, 32 * c)
                            nc.tensor.matmul(
                                out=sum_ps[row, :], lhsT=onesc[:],
                                rhs=tt[:, 0:QC], start=True, stop=False,
                                tile_position=tp)
                            nc.tensor.matmul(
                                out=sum_ps[row, :], lhsT=onesc[:],
                                rhs=tt[:, QC:2 * QC], start=False, stop=False,
                                tile_position=tp)
                            nc.tensor.matmul(
                                out=sum_ps[row, o0:QC], lhsT=onesc[:],
                                rhs=pp[:, o0:QC], start=False, stop=False,
                                tile_position=tp)
                            nc.tensor.matmul(
                                out=sum_ps[row, o1:QC], lhsT=onesc[:],
                                rhs=pp[:, QC + o1:2 * QC], start=False,
                                stop=True, tile_position=tp)
                            deferred.append(stage_a)
                            deferred.append(lambda: None)
                            deferred.append(stage_b)
                            deferred.append(lambda: None)
                            deferred.append(stage_c)

                        pending_ones.append(ones_chain)

                    # out^T accumulation, V stationary
                    nc.tensor.matmul(
                        out=o_ps[:, o0:QC],
                        lhsT=v_sb[:, D * t0:D * (t0 + 1)],
                        rhs=pp[:, o0:QC],
                        start=(j == 0), stop=False)
                    nc.tensor.matmul(
                        out=o_ps[:, o1:QC],
                        lhsT=v_sb[:, D * t1:D * (t1 + 1)],
                        rhs=pp[:, QC + o1:2 * QC],
                        start=False, stop=(j == npairs - 1))

    while pending_ones:
        pending_ones.pop(0)()
    while deferred:
        deferred.pop(0)()


_CACHED_NC = None


def build_program():
    global _CACHED_NC
    if _CACHED_NC is not None:
        return _CACHED_NC
    nc = bacc.Bacc("TRN2", target_bir_lowering=False, debug=False,
                   num_devices=NCORES)
    qt = nc.dram_tensor("qt", [HEADS_PER_CORE, D, S], BF16,
                        kind="ExternalInput").ap()
    kt = nc.dram_tensor("kt", [PAIRS_PER_CORE, D, S], BF16,
                        kind="ExternalInput").ap()
    v = nc.dram_tensor("v", [PAIRS_PER_CORE, 128, NKT * D], BF16,
                       kind="ExternalInput").ap()
    recd = nc.dram_tensor("recd", [HEADS_PER_CORE, NQC, QC], F32,
                          kind="Internal").ap()
    ot = nc.dram_tensor("ot", [HEADS_PER_CORE, D, S], BF16,
                        kind="ExternalOutput").ap()
    with tile.TileContext(nc) as tc:
        emit_core_program(tc, qt, kt, v, recd, ot)
    nc.compile()
    _CACHED_NC = nc
    return nc


def shard_inputs(query, key, value):
    """Full inputs -> list of 8 per-core in_maps (host-side relayout + bf16
    cast; halves the HBM input traffic and keeps the PE in bf16)."""
    import ml_dtypes
    bf16 = ml_dtypes.bfloat16
    query = np.asarray(query, dtype=np.float32).astype(bf16)
    key = np.asarray(key, dtype=np.float32).astype(bf16)
    value = np.asarray(value, dtype=np.float32).astype(bf16)

    # Q: [S,B,HQ,D] -> [B*HKV, G, D, S]
    qtall = np.ascontiguousarray(
        query.reshape(S, B, HKV, G, D).transpose(1, 2, 3, 4, 0)
    ).reshape(NPAIRS, G, D, S)
    # K: [S,B,HKV,D] -> [B*HKV, D, S]
    ktall = np.ascontiguousarray(
        key.transpose(1, 2, 3, 0)).reshape(NPAIRS, D, S)
    # V: [S,B,HKV,D] -> [B*HKV, k_local=128, NKT*D]
    vall = np.ascontiguousarray(
        value.reshape(NKT, 128, B, HKV, D).transpose(2, 3, 1, 0, 4)
    ).reshape(NPAIRS, 128, NKT * D)

    in_maps = []
    for c in range(NCORES):
        p0 = PAIRS_PER_CORE * c
        p1 = p0 + PAIRS_PER_CORE
        in_maps.append({
            "qt": np.ascontiguousarray(qtall[p0:p1].reshape(HEADS_PER_CORE, D, S)),
            "kt": np.ascontiguousarray(ktall[p0:p1]),
            "v": np.ascontiguousarray(vall[p0:p1]),
        })
    return in_maps


def unshard_output(results):
    """8 per-core {'ot': [8, D, S]} -> full [S, B, HQ, D]."""
    ot = np.stack([np.asarray(r["ot"], dtype=np.float32) for r in results])
    ot = ot.reshape(B, HKV, G, D, S)                   # pairs major -> b, hkv
    out = np.ascontiguousarray(ot.transpose(4, 0, 1, 2, 3))  # [S,B,HKV,G,D]
    return out.reshape(S, B, HQ, D)


def kernel(query, key, value, _trace=False, _return_bkr=False):
    nc = build_program()
    in_maps = shard_inputs(query, key, value)
    bkr = bass_utils.run_bass_kernel_spmd(
        nc, in_maps, core_ids=list(range(NCORES)), trace=_trace)
    out = unshard_output(bkr.results)
    if _return_bkr:
        return out, bkr
    return out


if __name__ == "__main__":
    q = np.random.randn(S, B, HQ, D).astype(np.float32)
    k = np.random.randn(S, B, HKV, D).astype(np.float32)
    vv = np.random.randn(S, B, HKV, D).astype(np.float32)
    o = kernel(q, k, vv)
    print("out", o.shape, o.dtype, float(np.abs(o).max()))
